# revision 22
# baseline (speedup 1.0000x reference)
"""Trainium2 Bass kernel for nn_Discriminator_80195629351349.

Pairwise-column MLP discriminator over k-space columns.

Math (matching the jax reference):
  F[b, w, ch] = |kspace[b, c, h, w]|  (ch = c*H + h)
  Pq = Fq @ W1[:, :CH].T ;  Pa = Fa @ W1[:, CH:].T          [B, W, 18]
  out[b, wi, wc] = sigmoid(W4 @ r3 + b4),  r3 = relu-chain of
                   relu(Pq[wi] + Pa[wc] + b1) through W2, W3
  heat[b, wi] = sum_wc out[b, wi, wc] * cmask[b, wc] / denom[b]
  result[b, h, w] = heat[b, w] if acquiring_mask[b, w] > 0 else 0

Sharding: 8 cores = (batch b in 0..3) x (wc half s in 0..1).

Column-grouped streaming: each core's wc columns are split into two
groups (A = wi slots + first half, B = second half). Group A streams
first; once its W1 projection finishes, its whole pair-MLP tail (pa4
replication, h1..W4) runs on DVE/ACT/PE *underneath* group B's DMA
stream. Both groups' sigmoids are deferred until after the last sqrt so
the ACT engine pays exactly two table loads (sqrt set, sigmoid set).
Everything on the wire is bf16, packed host-side partition-major so
every DMA line is one contiguous descriptor per partition; re/im are
separate planes so the square/add ops run in the DVE's dense 2x mode.
The device ships the raw [4, NL*(NWA+NWB)] sigmoid block; the host sums
over wc (skipping padding columns) and divides by denom.
"""

import math
import os

import numpy as np
import ml_dtypes

B, C, H, W = 4, 8, 384, 384
CH = C * H            # 3072 features per column
P = 128               # SBUF partitions
KT = CH // P          # 24 contraction tiles
CHANS = 18            # MLP width
NCORES = 8
BF16 = ml_dtypes.bfloat16

_prog_cache: dict = {}
LAST_RESULTS = None   # BassKernelResults of the most recent run (for test.py)


def _cst_layout(NWC: int, NS: int):
    """Column offsets in the bf16 constant block [128, CW]."""
    o = {}
    o["w1cat"] = 0                   # KT x 50 cols (Pa@0:18, Pq@32:50)
    o["w1end"] = KT * 50
    o["sel"] = o["w1end"]            # [0:18, 128] quadrant replicator
    o["sel32"] = o["sel"] + P        # [32:50, 128] same eye, rows 32:50
    o["b1p"] = o["sel32"] + P        # [1, 128] row 0: b1 quadrant pattern
    o["one"] = o["b1p"] + P          # [1, NWC] row 0: ones
    o["b2"] = o["one"] + NWC
    o["b3"] = o["b2"] + 1
    o["b4"] = o["b3"] + 1            # [4, 1]
    o["csel_end"] = o["b4"] + 1      # end of the selector/bias piece
    o["w2"] = o["csel_end"]          # [128, 128] block-diag
    o["w3"] = o["w2"] + P
    o["w4"] = o["w3"] + P            # [128, 4]
    o["end"] = o["w4"] + 4
    return o


def _build_program(NWC: int, NL: int):
    """SPMD Bass/Tile program for one core.

    NWC: wc (acquired) columns this core handles.
    NL:  wi slots per partition-quadrant (total wi slots = 4*NL).
    """
    import concourse.bass as bass
    import concourse.tile as tile
    from concourse import bacc, mybir

    f32 = mybir.dt.float32
    bf16 = mybir.dt.bfloat16
    NS = 4 * NL                   # wi slots
    NWA = (NWC + 1) // 2          # group A wc columns
    NWB = NWC - NWA               # group B wc columns
    NCA = NS + NWA                # group A columns per k-tile
    NCB = NWB
    NFA = NL * NWA
    NFB = NL * NWB
    NF = NFA + NFB
    CH_A = [2, 4, 4, 4, 5, 5]     # k-tiles per A chunk (ramped start)
    CH_B = [6, 6, 6, 6]
    assert sum(CH_A) == KT and sum(CH_B) == KT
    o = _cst_layout(NWC, NS)
    CW = o["end"]

    nc = bacc.Bacc("TRN2", debug=False)

    # ---- DRAM I/O (plane 0 = re, plane 1 = im; k-tile-major cols) ----
    ksA = nc.dram_tensor("ksA", [P, 2, KT * NCA], bf16, kind="ExternalInput")
    ksB = nc.dram_tensor("ksB", [P, 2, KT * NCB], bf16, kind="ExternalInput")
    cst = nc.dram_tensor("cst", [P, CW], bf16, kind="ExternalInput")
    hp = nc.dram_tensor("hp", [4, NF], bf16, kind="ExternalOutput")

    AF = mybir.ActivationFunctionType
    ALU = mybir.AluOpType

    with tile.TileContext(nc) as tc:
        with (
            tc.tile_pool(name="consts", bufs=1) as consts,
            tc.tile_pool(name="kdata", bufs=1) as kdata,
            tc.tile_pool(name="sq", bufs=1) as sqp,
            tc.tile_pool(name="m2", bufs=1) as m2p,
            tc.tile_pool(name="feat", bufs=1) as feat,
            tc.tile_pool(name="mlp", bufs=1) as mlp,
            tc.tile_pool(name="psum", bufs=1) as _unused,
            tc.tile_pool(name="ps", bufs=1, space="PSUM") as psp,
        ):
            # table-priming tile: ready at t0 so ACT loads the sqrt set
            # while the first k-space chunk is still in flight
            prim = mlp.tile([1, 2], f32, tag="prim")
            nc.gpsimd.memset(prim, 1.0)
            nc.scalar.sqrt(prim[:, 1:2], prim[:, 0:1])

            cst_s = consts.tile([P, CW], bf16, tag="cst")
            b2_s = cst_s[:, o["b2"]:o["b2"] + 1]
            b3_s = cst_s[:, o["b3"]:o["b3"] + 1]
            b4_s = cst_s[0:4, o["b4"]:o["b4"] + 1]

            # ---- DMA schedule: every queue leads with A chunks; w1 and
            # the selector piece ride mid-queue; B lands last ----
            fA = feat.tile([P, KT * NCA], bf16, tag="fA")
            fB = feat.tile([P, KT * NCB], bf16, tag="fB")
            psum1A = psp.tile([50, NCA], f32, tag="p1A")
            psum1B = psp.tile([50, NCB], f32, tag="p1B")
            kAo = [sum(CH_A[:i]) for i in range(len(CH_A) + 1)]
            kBo = [sum(CH_B[:i]) for i in range(len(CH_B) + 1)]
            kcsA, kcsB = {}, {}

            def ksadma(eng, ci):
                CL = CH_A[ci] * NCA
                kc = kdata.tile([P, 2, CL], bf16, tag=f"kA{ci}")
                eng.dma_start(out=kc,
                              in_=ksA[:, :, kAo[ci] * NCA:kAo[ci + 1] * NCA])
                kcsA[ci] = kc

            def ksbdma(eng, ci):
                CL = CH_B[ci] * NCB
                kc = kdata.tile([P, 2, CL], bf16, tag=f"kB{ci}")
                eng.dma_start(out=kc,
                              in_=ksB[:, :, kBo[ci] * NCB:kBo[ci + 1] * NCB])
                kcsB[ci] = kc

            W1H = 600
            ksadma(nc.scalar, 0)
            ksadma(nc.sync, 1)
            ksadma(nc.gpsimd, 2)
            nc.scalar.dma_start(out=cst_s[:, o["sel"]:o["csel_end"]],
                                in_=cst[:, o["sel"]:o["csel_end"]])
            nc.sync.dma_start(out=cst_s[:, 0:W1H], in_=cst[:, 0:W1H])
            nc.gpsimd.dma_start(out=cst_s[:, W1H:o["w1end"]],
                                in_=cst[:, W1H:o["w1end"]])
            ksadma(nc.scalar, 3)
            ksadma(nc.sync, 4)
            ksadma(nc.gpsimd, 5)
            nc.scalar.dma_start(out=cst_s[:, o["w2"]:], in_=cst[:, o["w2"]:])
            ksbdma(nc.sync, 0)
            ksbdma(nc.gpsimd, 1)
            ksbdma(nc.scalar, 2)
            ksbdma(nc.sync, 3)

            # ---- group A stream: sq/add (DVE), sqrt (ACT), W1 (PE) ----
            first_k = True
            for ci in range(len(CH_A)):
                CL = CH_A[ci] * NCA
                kc = kcsA[ci]
                sq = sqp.tile([P, 2, CL], bf16, tag=f"sqA{ci}")
                nc.vector.tensor_mul(sq, kc, kc)
                m2 = m2p.tile([P, CL], bf16, tag=f"m2A{ci}")
                nc.vector.tensor_add(m2, sq[:, 0, :], sq[:, 1, :])
                fc = fA[:, kAo[ci] * NCA:kAo[ci + 1] * NCA]
                nc.scalar.sqrt(fc, m2)
                for k in range(kAo[ci], kAo[ci + 1]):
                    nc.tensor.matmul(
                        out=psum1A,
                        lhsT=cst_s[:, 50 * k:50 * (k + 1)],
                        rhs=fA[:, k * NCA:(k + 1) * NCA],
                        start=first_k,
                        stop=(k == KT - 1))
                    first_k = False

            # ---- tail A head: cast, pa4A (+b1 rank-1), pq4p ----
            paqA = mlp.tile([50, NCA], bf16, tag="paqA")
            nc.vector.tensor_copy(paqA, psum1A)
            bf32 = mlp.tile([P, 2], f32, tag="bf32")
            nc.vector.tensor_copy(bf32, cst_s[:, o["b2"]:o["b2"] + 2])
            tailA = psp.tile([P, NWA + 2 * NFA], f32, tag="tailA")
            pa4A = tailA[:, 0:NWA]
            ps2A = tailA[:, NWA:NWA + NFA]
            ps3A = tailA[:, NWA + NFA:NWA + 2 * NFA]
            nc.tensor.matmul(out=pa4A,
                             lhsT=cst_s[0:CHANS, o["sel"]:o["sel"] + P],
                             rhs=paqA[0:CHANS, NS:NCA], start=True, stop=False)
            nc.tensor.matmul(out=pa4A,
                             lhsT=cst_s[0:1, o["b1p"]:o["b1p"] + P],
                             rhs=cst_s[0:1, o["one"]:o["one"] + NWA],
                             start=False, stop=True)
            # pq4p[128, NL]: full-width j=3 selector first, then the three
            # 32-wide slices of sel32 overwrite quadrants 0..2
            pq4p = psp.tile([P, NL], f32, tag="pq4p")
            nc.tensor.matmul(
                out=pq4p, lhsT=cst_s[32:32 + CHANS, o["sel32"]:o["sel32"] + P],
                rhs=paqA[32:32 + CHANS, 3 * NL:4 * NL], start=True, stop=True)
            for j in range(3):
                nc.tensor.matmul(
                    out=pq4p[32 * j:32 * (j + 1), :],
                    lhsT=cst_s[32:32 + CHANS,
                               o["sel32"] + 32 * j:o["sel32"] + 32 * (j + 1)],
                    rhs=paqA[32:32 + CHANS, j * NL:(j + 1) * NL],
                    start=True, stop=True)

            # ---- tail A body on DVE/PE/ACT (B still streaming) ----
            h1A = mlp.tile([P, NFA], bf16, tag="h1A")
            for lw in range(NL):
                nc.vector.tensor_scalar(
                    out=h1A[:, lw * NWA:(lw + 1) * NWA], in0=pa4A,
                    scalar1=pq4p[:, lw:lw + 1], scalar2=0.0,
                    op0=ALU.add, op1=ALU.max)
            nc.tensor.matmul(out=ps2A, lhsT=cst_s[:, o["w2"]:o["w2"] + P],
                             rhs=h1A, start=True, stop=True)
            h2A = mlp.tile([P, NFA], bf16, tag="h2A")
            nc.vector.tensor_scalar(out=h2A, in0=ps2A,
                                    scalar1=bf32[:, 0:1], scalar2=0.0,
                                    op0=ALU.add, op1=ALU.max)
            nc.tensor.matmul(out=ps3A, lhsT=cst_s[:, o["w3"]:o["w3"] + P],
                             rhs=h2A, start=True, stop=True)
            h3A = mlp.tile([P, NFA], bf16, tag="h3A")
            nc.scalar.activation(out=h3A, in_=ps3A, func=AF.Relu,
                                 bias=b3_s, scale=1.0)
            psum4 = psp.tile([4, NF], f32, tag="ps4")
            nc.tensor.matmul(out=psum4[:, 0:NFA],
                             lhsT=cst_s[:, o["w4"]:o["w4"] + 4],
                             rhs=h3A, start=True, stop=True)

            # ---- group B stream ----
            first_k = True
            for ci in range(len(CH_B)):
                CL = CH_B[ci] * NCB
                kc = kcsB[ci]
                sq = sqp.tile([P, 2, CL], bf16, tag=f"sqB{ci}")
                nc.vector.tensor_mul(sq, kc, kc)
                m2 = m2p.tile([P, CL], bf16, tag=f"m2B{ci}")
                nc.vector.tensor_add(m2, sq[:, 0, :], sq[:, 1, :])
                fc = fB[:, kBo[ci] * NCB:kBo[ci + 1] * NCB]
                nc.scalar.sqrt(fc, m2)
                for k in range(kBo[ci], kBo[ci + 1]):
                    nc.tensor.matmul(
                        out=psum1B,
                        lhsT=cst_s[:, 50 * k:50 * (k + 1)],
                        rhs=fB[:, k * NCB:(k + 1) * NCB],
                        start=first_k,
                        stop=(k == KT - 1))
                    first_k = False

            # swap ACT to the sigmoid set (relu lives there too); anchored
            # on the last B sqrt so it cannot be hoisted earlier
            nc.scalar.activation(
                out=prim[:, 1:2],
                in_=fB[0:1, KT * NCB - 1:KT * NCB], func=AF.Sigmoid)

            # ---- tail B ----
            paqB = mlp.tile([CHANS, NCB], bf16, tag="paqB")
            nc.vector.tensor_copy(paqB, psum1B[0:CHANS, :])
            tailB = psp.tile([P, NWB + 2 * NFB], f32, tag="tailB")
            pa4B = tailB[:, 0:NWB]
            ps2B = tailB[:, NWB:NWB + NFB]
            ps3B = tailB[:, NWB + NFB:NWB + 2 * NFB]
            nc.tensor.matmul(out=pa4B,
                             lhsT=cst_s[0:CHANS, o["sel"]:o["sel"] + P],
                             rhs=paqB, start=True, stop=False)
            nc.tensor.matmul(out=pa4B,
                             lhsT=cst_s[0:1, o["b1p"]:o["b1p"] + P],
                             rhs=cst_s[0:1, o["one"]:o["one"] + NWB],
                             start=False, stop=True)
            h1B = mlp.tile([P, NFB], bf16, tag="h1B")
            for lw in range(NL):
                nc.vector.tensor_scalar(
                    out=h1B[:, lw * NWB:(lw + 1) * NWB], in0=pa4B,
                    scalar1=pq4p[:, lw:lw + 1], scalar2=0.0,
                    op0=ALU.add, op1=ALU.max)
            nc.tensor.matmul(out=ps2B, lhsT=cst_s[:, o["w2"]:o["w2"] + P],
                             rhs=h1B, start=True, stop=True)
            h2B = mlp.tile([P, NFB], bf16, tag="h2B")
            nc.vector.tensor_scalar(out=h2B, in0=ps2B,
                                    scalar1=bf32[:, 0:1], scalar2=0.0,
                                    op0=ALU.add, op1=ALU.max)
            nc.tensor.matmul(out=ps3B, lhsT=cst_s[:, o["w3"]:o["w3"] + P],
                             rhs=h2B, start=True, stop=True)
            h3B = mlp.tile([P, NFB], bf16, tag="h3B")
            nc.scalar.activation(out=h3B, in_=ps3B, func=AF.Relu,
                                 bias=b3_s, scale=1.0)
            nc.tensor.matmul(out=psum4[:, NFA:NF],
                             lhsT=cst_s[:, o["w4"]:o["w4"] + 4],
                             rhs=h3B, start=True, stop=True)

            # ---- both sigmoids (single table swap already done) ----
            scr = mlp.tile([4, NF], bf16, tag="scr")
            nc.scalar.activation(out=scr[:, 0:NFA], in_=psum4[:, 0:NFA],
                                 func=AF.Sigmoid, bias=b4_s, scale=1.0)
            nc.scalar.activation(out=scr[:, NFA:NF], in_=psum4[:, NFA:NF],
                                 func=AF.Sigmoid, bias=b4_s, scale=1.0)
            nc.sync.dma_start(out=hp[:], in_=scr)

    nc.finalize()
    return nc


def _run_sim(nc, in_maps):
    """CoreSim (CPU instruction simulator) path for local dev testing."""
    from concourse.bass_interp import MultiCoreSim
    from concourse.bass_utils import BassKernelResults

    sim = MultiCoreSim(nc, num_cores=len(in_maps))
    for core_id, core in sim.cores.items():
        for name, arr in in_maps[core_id].items():
            core.tensor(name)[:] = arr
    sim.simulate()
    results = [
        {"hp": np.array(sim.cores[i].tensor("hp"))} for i in range(len(in_maps))
    ]
    return BassKernelResults(results=results, instructions_and_trace=None,
                             profile_json=None, exec_time_ns=None)


def _mask_geometry(acquired_mask, acquiring_mask):
    """Replicates the reference's left/right/cmask/denom logic exactly."""
    am = np.asarray(acquired_mask, np.float32)
    qm = np.asarray(acquiring_mask, np.float32)
    mid = W // 2
    right = mid + np.argmax(am[:, mid:] < 1.0, axis=1)
    left = np.argmax(am[:, :mid][:, ::-1] < 1.0, axis=1) + 1
    cols = np.arange(W)
    cmask = (cols[None, :] >= left[:, None]) & (cols[None, :] < right[:, None])
    denom = (right - left).astype(np.float32)
    active = [np.nonzero(qm[b] > 0)[0] for b in range(B)]
    return left.astype(int), right.astype(int), cmask, denom, active


def kernel(acquired_kspace, acquiring_kspace, acquired_mask, acquiring_mask,
           W1, b1, W2, b2, W3, b3, W4, b4):
    global LAST_RESULTS
    from concourse.bass_utils import run_bass_kernel_spmd

    acquired_kspace = np.asarray(acquired_kspace, np.float32)
    acquiring_kspace = np.asarray(acquiring_kspace, np.float32)
    W1 = np.asarray(W1, np.float32)
    b1 = np.asarray(b1, np.float32)
    W2 = np.asarray(W2, np.float32)
    b2 = np.asarray(b2, np.float32)
    W3 = np.asarray(W3, np.float32)
    b3 = np.asarray(b3, np.float32)
    W4 = np.asarray(W4, np.float32)
    b4 = np.asarray(b4, np.float32)

    left, right, cmask, denom, active = _mask_geometry(acquired_mask, acquiring_mask)

    nmax = max(len(a) for a in active)
    out = np.zeros((B, H, W), np.float32)
    if nmax == 0:
        return out

    span = max(int((right - left).max()), 1)
    NL = max(1, math.ceil(nmax / 4))          # wi slots per quadrant
    NWC = max(1, math.ceil(span / 2))         # wc columns per core
    NS = 4 * NL
    NWA = (NWC + 1) // 2
    NWB = NWC - NWA
    NCA = NS + NWA
    assert NL * NWC <= 512, (NL, NWC)
    o = _cst_layout(NWC, NS)
    CW = o["end"]

    # ---- shared constant block [128, CW] bf16 ----
    W1q, W1a = W1[:, :CH], W1[:, CH:]
    cstv = np.zeros((P, CW), np.float32)
    # w1cat: per k-tile 50 cols; W1a_k at 0:18, W1q_k at 32:50 so both
    # PSUM row groups are 32-aligned for engine reads
    w1q_t = W1q.T.reshape(KT, P, CHANS)   # [k, p, i]
    w1a_t = W1a.T.reshape(KT, P, CHANS)
    w1cat = np.zeros((KT, P, 50), np.float32)
    w1cat[:, :, 0:CHANS] = w1a_t
    w1cat[:, :, 32:32 + CHANS] = w1q_t
    cstv[:, :KT * 50] = w1cat.transpose(1, 0, 2).reshape(P, -1)
    eye = np.eye(CHANS, dtype=np.float32)
    for j in range(4):
        sl = slice(32 * j, 32 * j + CHANS)
        cstv[:CHANS, o["sel"] + 32 * j:o["sel"] + 32 * j + CHANS] = eye
        cstv[32:32 + CHANS, o["sel32"] + 32 * j:
             o["sel32"] + 32 * j + CHANS] = eye
        cstv[sl, o["w2"] + 32 * j:o["w2"] + 32 * j + CHANS] = W2.T
        cstv[sl, o["w3"] + 32 * j:o["w3"] + 32 * j + CHANS] = W3.T
        cstv[sl, o["w4"] + j] = W4[0]
        cstv[sl, o["b2"]] = b2
        cstv[sl, o["b3"]] = b3
        cstv[0, o["b1p"] + 32 * j:o["b1p"] + 32 * j + CHANS] = b1
    cstv[0, o["one"]:o["one"] + NWC] = 1.0
    cstv[:4, o["b4"]] = float(b4[0])
    cstv = cstv.astype(BF16)

    # ---- per-core packed k-space, split into column groups A and B ----
    in_maps = []
    meta = []
    for b in range(B):
        aw = active[b]
        awp = np.zeros(NS, np.int64)
        if len(aw):
            awp[:len(aw)] = aw
            awp[len(aw):] = aw[0]
        qcols = acquiring_kspace[b].reshape(CH, W, 2)[:, awp, :]
        for s in range(2):
            w0 = int(left[b]) + s * NWC
            w1e = max(min(w0 + NWC, W), w0)
            nreal = w1e - w0
            acols = np.zeros((CH, NWC, 2), np.float32)
            if nreal > 0:
                acols[:, :nreal, :] = (
                    acquired_kspace[b].reshape(CH, W, 2)[:, w0:w1e, :])
            combA = np.concatenate([qcols, acols[:, :NWA, :]], axis=1)
            combB = acols[:, NWA:, :]
            ksvA = combA.reshape(KT, P, NCA, 2).transpose(1, 3, 0, 2)
            ksvB = combB.reshape(KT, P, NWB, 2).transpose(1, 3, 0, 2)
            in_maps.append(dict(
                ksA=np.ascontiguousarray(
                    ksvA.reshape(P, 2, KT * NCA)).astype(BF16),
                ksB=np.ascontiguousarray(
                    ksvB.reshape(P, 2, KT * NWB)).astype(BF16),
                cst=cstv))
            meta.append((b, s, NWC - nreal))
    key = (NWC, NL)
    if key not in _prog_cache:
        _prog_cache[key] = _build_program(NWC, NL)
    nc = _prog_cache[key]

    trace = bool(int(os.environ.get("CABSK_TRACE", "0")))
    tmpdir = os.environ.get("CABSK_TMPDIR") or None
    if tmpdir:
        import tempfile
        tmpdir = tempfile.mkdtemp(dir=tmpdir)
    if os.environ.get("CABSK_SIM", "0") == "1":
        res = _run_sim(nc, in_maps)
    else:
        res = run_bass_kernel_spmd(nc, in_maps, core_ids=list(range(NCORES)),
                                   trace=trace, tmpdir=tmpdir)
    LAST_RESULTS = res

    # ---- host epilogue: per-column sums, skipping padding columns ----
    NFA = NL * NWA
    heat = np.zeros((B, W), np.float32)
    for ci, (b, s, npad) in enumerate(meta):
        hpv = np.asarray(res.results[ci]["hp"], np.float32)   # [4, NF]
        sA = hpv[:, :NFA].reshape(4, NL, NWA)
        sB = hpv[:, NFA:].reshape(4, NL, NWB)
        # padding columns sit at the end of group B (nreal >= NWA here)
        padB = min(npad, NWB)
        padA = npad - padB
        hsum = (sA[:, :, :NWA - padA].sum(axis=2)
                + sB[:, :, :NWB - padB].sum(axis=2))
        aw = active[b]
        d = denom[b] if denom[b] != 0 else 1.0
        for t in range(len(aw)):
            heat[b, aw[t]] += hsum[t // NL, t % NL] / d
    out[:] = heat[:, None, :]
    return out
